# revision 42
# baseline (speedup 1.0000x reference)
"""Trainium2 Bass kernel for nn_BaseLinearSSM (chunked formulation, v2).

y[b,t] = Re(C @ x_{t+1}) + D @ u[b,t] + bias,  x_{t+1} = A x_t + B u_t  (complex A,B,C)

Strategy (chunk length L=8, NK=T/L=256 chunks):
  Host (fp64): eigendecompose A = V diag(w) V^-1, Bt = V^-1 B, Ct = C V.
  Precompute:
    Pt_j = diag(w^(L-1-j)) Bt          [N,IN]  (chunk input aggregation)
    Qt_j = Ct diag(w^(j+1))            [OUT,N] (chunk boundary -> outputs)
    K_d  = Re(C A^d B), K_0 += D       [OUT,IN] real (within-chunk causal conv)
  Device (per core, batch-sharded 2 of 16; fp16 data, fp32 PSUM/scan state):
    phase 1: vt_k = sum_j Pt_j u_{kL+j}  (matmuls, PSUM; m0 is DMA-paced and
             doubles as the PE p-state ramp — no separate warmup)
    phase 2: S_k = w^L S_{k-1} + vt_k on the CHUNK axis, DVE-only (gpsimd
             TT concurrency slows DVE ~3.3x, so it is not used at all).
             Ops are batched [128,1024] over packed v=[vr|vi] with tables
             ckck=[ck|ck], skpm=[sk|-sk], rr=[rho|rho] and ONE 1024-wide
             scan per m:
               t_a = ckck * v;  t_b = skpm-crossed * v;  g = t_a + t_b
               z   = scan(rr, g)          (zeros in rr reset at col 256/512/768)
               t_c = ckck * z;  t_d = skpm-crossed * z;  S = t_c + t_d = [Sr|Si]
    phase 3: y_{kL+j} = Re(Qt_j beta_k) + sum_d K_d u_{kL+j-d}
             conv(j) interleaved into phase-1 DMA bubbles; boundary matmuls
             m-OUTER so S3 (latest) is consumed at the very end; evictions
             stream per-j inside the last m-block on scalar+vector.
  Time is laid out (j, b, k) so every matmul has 512 contiguous columns.
  DMA: sync q = u_j0, Pt0, u_j1..7, Pt1..3, y-out; scalar q = K, tables,
  Qt (deferred) — ordered so each consumer's data lands just in time.
"""

import sys

import numpy as np

if "/opt/trn_rl_repo" not in sys.path:
    sys.path.insert(0, "/opt/trn_rl_repo")

BATCH, T, IN, OUT, N = 16, 2048, 128, 128, 512
NCORES = 8
BLOCAL = BATCH // NCORES   # 2
L = 8                      # chunk length
NK = T // L                # 256 chunks per batch element
NKB = BLOCAL * NK          # 512 chunk-columns per core (b-major)
NT = N // 128              # 4 partition tiles over the state dim
COLS = BLOCAL * T          # 4096
W2 = 2 * NKB               # 1024: width of packed [r|i] phase-2 tiles

# blob (fp16) column layout:
#   u [0:4096] | Pt (4 x 2048) | K (1024) | tables (4 x 3072) | Qt (8192)
UW = L * NKB               # 4096
PW = 2 * L * 128           # 2048 per m  (ri-major, j-minor, 128 each)
KW = L * 128               # 1024
TRW = 3 * W2               # 3072 per m: ckck | skpm | rr
QW = L * 2 * NT * 128      # 8192
O_PT = UW
O_K = O_PT + NT * PW
O_TR = O_K + KW
O_QT = O_TR + NT * TRW
W16 = O_QT + QW            # 33792

LAST_RESULT = None
_NC_CACHE = None


def _build_nc():
    from concourse import bass, mybir
    from concourse import tile

    f32 = mybir.dt.float32
    f16 = mybir.dt.float16
    op = mybir.AluOpType

    nc = bass.Bass("TRN2", target_bir_lowering=False, debug=False)

    blob = nc.dram_tensor("blob", [128, W16], f16, kind="ExternalInput")
    yout = nc.dram_tensor("y", [OUT, COLS], f16, kind="ExternalOutput")

    with tile.TileContext(nc) as tc:
        with (
            tc.tile_pool(name="const", bufs=1) as cpool,
            tc.tile_pool(name="vsb", bufs=4) as vpool,
            tc.tile_pool(name="tmp", bufs=2) as tpool,
            tc.tile_pool(name="gz", bufs=2) as gpool,
            tc.tile_pool(name="S", bufs=1) as spool,
            tc.tile_pool(name="ysb", bufs=4) as ypool_sb,
            tc.tile_pool(name="ps", bufs=1, space="PSUM") as pspool,
        ):
            b16 = cpool.tile([128, W16], f16)

            u_j = [b16[:, j * NKB:(j + 1) * NKB] for j in range(L)]
            ptT = [[[None] * L for _ in range(2)] for _ in range(NT)]
            for m in range(NT):
                for ri in range(2):
                    for j in range(L):
                        o = O_PT + m * PW + ri * L * 128 + j * 128
                        ptT[m][ri][j] = b16[:, o:o + 128]
            ktT = [b16[:, O_K + d * 128:O_K + (d + 1) * 128] for d in range(L)]
            ckck = [None] * NT
            skpm = [None] * NT
            rr = [None] * NT
            for m in range(NT):
                o = O_TR + m * TRW
                ckck[m] = b16[:, o:o + W2]
                skpm[m] = b16[:, o + W2:o + 2 * W2]
                rr[m] = b16[:, o + 2 * W2:o + 3 * W2]
            qtT = [[[None] * NT for _ in range(2)] for _ in range(L)]
            for j in range(L):
                for ri in range(2):
                    for m in range(NT):
                        o = O_QT + (j * 2 + ri) * NT * 128 + m * 128
                        qtT[j][ri][m] = b16[:, o:o + 128]

            # ---- DMA: two HW rings only (the gpsimd ring is a slow SWDGE
            # path and 3-way queue splitting starves everyone). Triggers
            # cost ~0.7us each on the issuing engine, so few, large pieces:
            #   sync q:   u_j0 | Pt0 | u_rest | Pt1 | Pt2 | Pt3, y-out later
            #   scalar q: K, tr0 up front; tr1, Qt halves, tr2/tr3
            #             interleaved with the v-copies (just-in-time)
            def tr_dma(m, eng=None):
                lo, hi = O_TR + m * TRW, O_TR + (m + 1) * TRW
                (eng or nc.scalar).dma_start(b16[:, lo:hi], blob[:, lo:hi])

            # u split across BOTH rings to beat the DMA rate ramp; phase-1 m0
            # starts only when it can run gap-free (a stall-y early matmul
            # burst provokes a HAM half-speed window)
            nc.sync.dma_start(b16[:, 0:UW // 2], blob[:, 0:UW // 2])
            h0 = O_PT + PW // 2
            nc.sync.dma_start(b16[:, O_PT:h0], blob[:, O_PT:h0])
            nc.sync.dma_start(b16[:, h0:O_PT + PW], blob[:, h0:O_PT + PW])
            nc.scalar.dma_start(b16[:, UW // 2:UW], blob[:, UW // 2:UW])
            nc.scalar.dma_start(b16[:, O_K:O_K + KW], blob[:, O_K:O_K + KW])
            tr_dma(0)  # m0 tables: after K (needed ~17us, lands ~15)
            for m in range(1, NT):
                lo, hi = O_PT + m * PW, O_PT + (m + 1) * PW
                nc.sync.dma_start(b16[:, lo:hi], blob[:, lo:hi])

            # PE p-state warmup on a memset tile, entirely inside the DMA
            # dead zone (no data dependencies); output discarded.
            wsrc = vpool.tile([128, NKB], f16, tag="warm", name="wsrc", bufs=1)
            nc.vector.memset(wsrc[:], 0.0)
            wp = pspool.tile([128, NKB], f32, tag="vt", bufs=2, name="warm")
            for wi in range(14):
                nc.tensor.matmul(wp[:], wsrc[:, 0:128], wsrc[:],
                                 start=(wi == 0), stop=(wi == 13))

            v_t = [None] * NT
            S_t = [None] * NT
            yps = {}

            def phase1(m, mid=None):
                # ri-major with the v-copy issued right after each half, so
                # the real part of v reaches the DVE ~1.3us earlier; `mid`
                # emits PE filler (conv) between the halves to cover Pt DMA
                v = vpool.tile([128, W2], f16, tag="v", name=f"v{m}")
                for ri in range(2):
                    vt = pspool.tile([128, NKB], f32, tag="vt", bufs=2,
                                     name=f"vt{m}{ri}")
                    for j in range(L):
                        nc.tensor.matmul(
                            vt[:], ptT[m][ri][j], u_j[j],
                            start=(j == 0), stop=(j == L - 1),
                        )
                    nc.scalar.copy(v[:, ri * NKB:(ri + 1) * NKB], vt[:])
                    if ri == 0 and mid:
                        mid()
                v_t[m] = v

            def phase2(m, split_first=False):
                ta = tpool.tile([128, W2], f16, tag="ta", name=f"ta{m}")
                tb = tpool.tile([128, W2], f16, tag="tb", name=f"tb{m}")
                tc_ = tpool.tile([128, W2], f16, tag="tc", name=f"tc{m}")
                td = tpool.tile([128, W2], f16, tag="td", name=f"td{m}")
                g = gpool.tile([128, W2], f16, tag="g", name=f"g{m}")
                z = gpool.tile([128, W2], f16, tag="z", name=f"z{m}")
                S = spool.tile([128, W2], f16, tag=f"S{m}", name=f"S{m}")
                v = v_t[m]
                V = nc.vector
                if split_first:
                    # vr-dependent halves first: vr lands ~1.4us before vi
                    V.tensor_tensor(ta[:, 0:NKB], ckck[m][:, 0:NKB],
                                    v[:, 0:NKB], op=op.mult)
                    V.tensor_tensor(tb[:, NKB:W2], skpm[m][:, NKB:W2],
                                    v[:, 0:NKB], op=op.mult)
                    V.tensor_tensor(ta[:, NKB:W2], ckck[m][:, NKB:W2],
                                    v[:, NKB:W2], op=op.mult)
                    V.tensor_tensor(tb[:, 0:NKB], skpm[m][:, 0:NKB],
                                    v[:, NKB:W2], op=op.mult)
                else:
                    V.tensor_tensor(ta[:], ckck[m], v[:], op=op.mult)
                    # t_b = [sk*vi | -sk*vr] (crossed halves)
                    V.tensor_tensor(tb[:, 0:NKB], skpm[m][:, 0:NKB],
                                    v[:, NKB:W2], op=op.mult)
                    V.tensor_tensor(tb[:, NKB:W2], skpm[m][:, NKB:W2],
                                    v[:, 0:NKB], op=op.mult)
                V.tensor_tensor(g[:], ta[:], tb[:], op=op.add)
                V.tensor_tensor_scan(z[:], rr[m], g[:], 0.0,
                                     op0=op.mult, op1=op.add)
                if m == NT - 1:
                    # demod split r-half-first: Sr3 unblocks the final bnd
                    # block ~1us before Si3 is needed
                    V.tensor_tensor(tc_[:, 0:NKB], ckck[m][:, 0:NKB],
                                    z[:, 0:NKB], op=op.mult)
                    V.tensor_tensor(td[:, 0:NKB], skpm[m][:, NKB:W2],
                                    z[:, NKB:W2], op=op.mult)
                    V.tensor_tensor(S[:, 0:NKB], tc_[:, 0:NKB],
                                    td[:, 0:NKB], op=op.add)
                    V.tensor_tensor(tc_[:, NKB:W2], ckck[m][:, NKB:W2],
                                    z[:, NKB:W2], op=op.mult)
                    V.tensor_tensor(td[:, NKB:W2], skpm[m][:, 0:NKB],
                                    z[:, 0:NKB], op=op.mult)
                    V.tensor_tensor(S[:, NKB:W2], tc_[:, NKB:W2],
                                    td[:, NKB:W2], op=op.add)
                else:
                    V.tensor_tensor(tc_[:], ckck[m], z[:], op=op.mult)
                    # t_d = [-sk*zi | sk*zr] (crossed halves)
                    V.tensor_tensor(td[:, 0:NKB], skpm[m][:, NKB:W2],
                                    z[:, NKB:W2], op=op.mult)
                    V.tensor_tensor(td[:, NKB:W2], skpm[m][:, 0:NKB],
                                    z[:, 0:NKB], op=op.mult)
                    V.tensor_tensor(S[:], tc_[:], td[:], op=op.add)
                S_t[m] = S

            # bnd uses ONE 511-col matmul per (j,ri,m) spanning both batch
            # halves; the batch-boundary column (k=0 of b1) picks up a bogus
            # carry-over, so its conv-only value is captured here and patched
            # back after the eviction cast.
            ccol = vpool.tile([128, L], f32, tag="ccol", name="ccol", bufs=1)

            def capture(j):
                nc.scalar.copy(ccol[:, j:j + 1], yps[j][:, NK:NK + 1])

            def conv(j, tag="y", cap=True):
                yps[j] = pspool.tile([128, NKB], f32, tag=tag,
                                     bufs=(6 if tag == "y" else 2),
                                     name=f"y{j}")
                for d in range(j + 1):
                    nc.tensor.matmul(
                        yps[j][:], ktT[d], u_j[j - d],
                        start=(d == 0), stop=False, skip_group_check=True,
                    )
                if cap:
                    capture(j)

            # ---- emission: p1/conv interleaved so conv fills DMA bubbles;
            # conv4..7 deferred into the bnd section to cover S-latency ----
            qh = O_QT + QW // 2
            phase1(0, mid=lambda: (conv(0, cap=False), conv(1, cap=False)))
            phase2(0, split_first=True)
            tr_dma(1)
            conv(2, cap=False)
            phase1(1, mid=lambda: conv(3, cap=False))
            phase2(1)
            nc.scalar.dma_start(b16[:, O_QT:qh], blob[:, O_QT:qh])  # Qt j0..3
            nc.scalar.dma_start(b16[:, qh:W16], blob[:, qh:W16])    # Qt j4..7
            tr_dma(2)
            phase1(2, mid=lambda: conv(4, cap=False))
            phase2(2)
            tr_dma(3)
            phase1(3, mid=lambda: conv(5, cap=False))
            phase2(3)
            # captures for j0..5 AFTER the last v-copy: a capture queued
            # between v-copies on the scalar ring delays them (WAR on conv)
            # and fragments the whole DVE chain
            for j in range(6):
                capture(j)

            # ---- boundary matmuls, m-outer (S3 needed only at the end) ----
            def bnd(j, m, ri, stop=False):
                S = S_t[m]
                a0 = ri * NKB
                nc.tensor.matmul(
                    yps[j][:, 1:NKB], qtT[j][ri][m],
                    S[:, a0:a0 + NKB - 1], start=False, stop=stop,
                    skip_group_check=True,
                )

            conv(6, tag="vt")   # vt PSUM banks are free after p1(3)'s copies
            for m in range(NT - 1):
                for j in range(L - 1):
                    for ri in range(2):
                        bnd(j, m, ri)
            # j=7 column deferred: conv(7) + its m0..2 bnds sit right before
            # the m3 block, filling the wait for S3
            conv(7, tag="vt")
            for m in range(NT - 1):
                for ri in range(2):
                    bnd(L - 1, m, ri)
            # last m: real parts first (Si3 lands after Sr3), then imag
            # j-by-j with eviction casts streaming right behind into ONE
            # ysb tile; y-out DMA in 3 pieces so the transfers overlap the
            # remaining casts.
            ysb = ypool_sb.tile([128, COLS], f16, tag="ysb", bufs=1)
            # batch-boundary columns (conv-only) written into ysb EARLY,
            # off the tail; the eviction casts below skip that column
            for j in range(L):
                nc.scalar.copy(ysb[:, j * NKB + NK:j * NKB + NK + 1],
                               ccol[:, j:j + 1])
            for j in range(L):
                bnd(j, NT - 1, 0)
            for j in range(L):
                bnd(j, NT - 1, 1, stop=True)
                o = j * NKB
                # each eviction split across BOTH engines: halves the per-j
                # cast latency so the y-out pieces can launch earlier
                nc.scalar.copy(ysb[:, o:o + NK], yps[j][:, 0:NK])
                nc.vector.tensor_copy(ysb[:, o + NK + 1:o + NKB],
                                      yps[j][:, NK + 1:NKB])
                # y-out in 4 pieces across BOTH rings so transfers overlap
                if j == 1:
                    nc.sync.dma_start(yout[:, 0:2 * NKB], ysb[:, 0:2 * NKB])
                if j == 4:
                    nc.scalar.dma_start(yout[:, 2 * NKB:5 * NKB],
                                        ysb[:, 2 * NKB:5 * NKB])
                if j == 6:
                    nc.sync.dma_start(yout[:, 5 * NKB:7 * NKB],
                                      ysb[:, 5 * NKB:7 * NKB])
            nc.scalar.dma_start(yout[:, 7 * NKB:COLS], ysb[:, 7 * NKB:COLS])
    _legalize_multi_waits(nc)
    return nc


def _legalize_multi_waits(nc):
    """This walrus build accepts a single sync wait per instruction; split
    any multi-wait instruction into same-engine single-wait NoOps + the
    original carrying the last wait (program order chains them)."""
    import bass_rust
    from concourse import mybir

    uid = [0]
    for fn in nc.m.functions:
        for bb in fn.blocks:
            insts = bb.instructions
            new = []
            changed = False
            for inst in insts:
                si = inst.sync_info
                if si is not None and len(si.on_wait) > 1:
                    waits = list(si.on_wait)
                    for w in waits[:-1]:
                        uid[0] += 1
                        new.append(mybir.InstNoOp(
                            name=f"mwsplit-{uid[0]}",
                            engine=inst.engine,
                            ins=[], outs=[],
                            sync_info=bass_rust.SyncInfo(on_wait=[w], on_update=[]),
                        ))
                    inst.sync_info = bass_rust.SyncInfo(
                        on_wait=[waits[-1]], on_update=list(si.on_update)
                    )
                    changed = True
                new.append(inst)
            if changed:
                bb.instructions = new


def _host_prep(A_re, A_im, B_re, B_im, C_re, C_im, D_w):
    """fp64 eigendecomposition + chunked-formulation weight/table layouts.
    Returns shared fp16 tail of the blob: [128, W16 - UW]."""
    A = A_re.astype(np.float64) + 1j * A_im.astype(np.float64)
    w, V = np.linalg.eig(A)
    Vinv = np.linalg.inv(V)
    Bt = Vinv @ (B_re.astype(np.float64) + 1j * B_im.astype(np.float64))
    Ct = (C_re.astype(np.float64) + 1j * C_im.astype(np.float64)) @ V

    Pt = np.stack([(w ** (L - 1 - j))[:, None] * Bt for j in range(L)])  # [L,N,IN]
    Qt = np.stack([Ct * (w ** (j + 1))[None, :] for j in range(L)])      # [L,OUT,N]
    K = np.empty((L, OUT, IN))
    Ad = np.eye(N, dtype=complex)
    Bc = B_re.astype(np.float64) + 1j * B_im.astype(np.float64)
    Cc = C_re.astype(np.float64) + 1j * C_im.astype(np.float64)
    for d in range(L):
        K[d] = (Cc @ Ad @ Bc).real
        Ad = A @ Ad
    K[0] += D_w.astype(np.float64)

    wL = w ** L
    rhoL = np.abs(wL)
    phi = np.angle(wL)
    kk = np.arange(NK)
    cosk = np.cos(np.outer(phi, kk + 1))  # [N, NK]
    sink = np.sin(np.outer(phi, kk + 1))

    parts = []
    for m in range(NT):
        sl = slice(m * 128, (m + 1) * 128)
        for Pp in (Pt.real, Pt.imag):
            for j in range(L):
                parts.append(np.ascontiguousarray(Pp[j].T[:, sl]))  # [IN, 128]
    for d in range(L):
        parts.append(np.ascontiguousarray(K[d].T))  # [IN, OUT]
    for m in range(NT):
        sl = slice(m * 128, (m + 1) * 128)
        ck = np.tile(cosk[sl], (1, BLOCAL))          # [128, NKB]
        sk = np.tile(sink[sl], (1, BLOCAL))
        rb = np.broadcast_to(rhoL[sl][:, None], (128, NKB)).copy()
        rb[:, NK] = 0.0  # reset scan state at second batch element
        rr2 = np.concatenate([rb, rb], axis=1)       # [128, W2]
        rr2[:, NKB] = 0.0  # reset crossing into the imag half
        parts.append(np.concatenate([ck, ck], axis=1))   # ckck
        parts.append(np.concatenate([sk, -sk], axis=1))  # skpm
        parts.append(rr2)                                # rr
    for j in range(L):
        for Qp in (Qt[j].real, -Qt[j].imag):
            QT = np.ascontiguousarray(Qp.T)  # [N, OUT]
            for m in range(NT):
                parts.append(QT[m * 128:(m + 1) * 128])
    shared = np.concatenate(parts, axis=1).astype(np.float16)
    assert shared.shape == (128, W16 - UW)
    return shared


def _ensure_axon_hooks():
    """Provide antenv.axon_hooks if the image lacks it (needed only for
    trace=True NTFF profiling; run path works without)."""
    import types
    try:
        from antenv import axon_hooks  # noqa: F401
        return
    except ImportError:
        pass
    try:
        import antenv
        mod = types.ModuleType("antenv.axon_hooks")
        _hook = [None]
        mod.set_axon_ntff_profile_hook = lambda h: _hook.__setitem__(0, h)
        mod.get_axon_ntff_profile_hook = lambda: _hook[0]
        sys.modules["antenv.axon_hooks"] = mod
        antenv.axon_hooks = mod
        if "/root/.axon_site" not in sys.path:
            sys.path.insert(0, "/root/.axon_site")
        from trn_agent_boot.trn_boot import _ntff_profile_via_ctypes
        h = _ntff_profile_via_ctypes("/opt/axon/libaxon_pjrt.so")
        if h is not None:
            mod.set_axon_ntff_profile_hook(h)
    except Exception:
        pass


def kernel(u, A_re, A_im, B_re, B_im, C_re, C_im, D_w, output_bias):
    global LAST_RESULT, _NC_CACHE
    from concourse import bass_utils

    _ensure_axon_hooks()

    u = np.asarray(u, dtype=np.float32)
    shared = _host_prep(
        np.asarray(A_re), np.asarray(A_im), np.asarray(B_re), np.asarray(B_im),
        np.asarray(C_re), np.asarray(C_im), np.asarray(D_w)
    )

    if _NC_CACHE is None:
        _NC_CACHE = _build_nc()
    nc = _NC_CACHE

    in_maps = []
    for c in range(NCORES):
        up = u[BLOCAL * c:BLOCAL * (c + 1)]           # [2, T, IN]
        uc = up.reshape(BLOCAL, NK, L, IN)            # t = k*L + j
        u_jk = np.ascontiguousarray(
            uc.transpose(3, 2, 0, 1).reshape(IN, L * NKB)
        ).astype(np.float16)                          # col = j*NKB + b*NK + k
        in_maps.append({"blob": np.concatenate([u_jk, shared], axis=1)})

    res = bass_utils.run_bass_kernel_spmd(nc, in_maps, core_ids=list(range(NCORES)))
    LAST_RESULT = res

    y = np.empty((BATCH, T, OUT), dtype=np.float32)
    for c in range(NCORES):
        yd = np.asarray(res.results[c]["y"], dtype=np.float32)  # [OUT, L*NKB]
        y[BLOCAL * c:BLOCAL * (c + 1)] = (
            yd.reshape(OUT, L, BLOCAL, NK).transpose(2, 3, 1, 0)
            .reshape(BLOCAL, T, OUT)
        )
    y += np.asarray(output_bias, dtype=np.float32)
    return y


# revision 45
# speedup vs baseline: 1.0722x; 1.0722x over previous
"""Trainium2 Bass kernel for nn_BaseLinearSSM (chunked formulation, v2).

y[b,t] = Re(C @ x_{t+1}) + D @ u[b,t] + bias,  x_{t+1} = A x_t + B u_t  (complex A,B,C)

Strategy (chunk length L=8, NK=T/L=256 chunks):
  Host (fp64): eigendecompose A = V diag(w) V^-1, Bt = V^-1 B, Ct = C V.
  Precompute:
    Pt_j = diag(w^(L-1-j)) Bt          [N,IN]  (chunk input aggregation)
    Qt_j = Ct diag(w^(j+1))            [OUT,N] (chunk boundary -> outputs)
    K_d  = Re(C A^d B), K_0 += D       [OUT,IN] real (within-chunk causal conv)
  Device (per core, batch-sharded 2 of 16; fp16 data, fp32 PSUM/scan state):
    phase 1: vt_k = sum_j Pt_j u_{kL+j}  (matmuls, PSUM; m0 is DMA-paced and
             doubles as the PE p-state ramp — no separate warmup)
    phase 2: S_k = w^L S_{k-1} + vt_k on the CHUNK axis, DVE-only (gpsimd
             TT concurrency slows DVE ~3.3x, so it is not used at all).
             Ops are batched [128,1024] over packed v=[vr|vi] with tables
             ckck=[ck|ck], skpm=[sk|-sk], rr=[rho|rho] and ONE 1024-wide
             scan per m:
               t_a = ckck * v;  t_b = skpm-crossed * v;  g = t_a + t_b
               z   = scan(rr, g)          (zeros in rr reset at col 256/512/768)
               t_c = ckck * z;  t_d = skpm-crossed * z;  S = t_c + t_d = [Sr|Si]
    phase 3: y_{kL+j} = Re(Qt_j beta_k) + sum_d K_d u_{kL+j-d}
             conv(j) interleaved into phase-1 DMA bubbles; boundary matmuls
             m-OUTER so S3 (latest) is consumed at the very end; evictions
             stream per-j inside the last m-block on scalar+vector.
  Time is laid out (j, b, k) so every matmul has 512 contiguous columns.
  DMA: sync q = u_j0, Pt0, u_j1..7, Pt1..3, y-out; scalar q = K, tables,
  Qt (deferred) — ordered so each consumer's data lands just in time.
"""

import sys

import numpy as np

if "/opt/trn_rl_repo" not in sys.path:
    sys.path.insert(0, "/opt/trn_rl_repo")

BATCH, T, IN, OUT, N = 16, 2048, 128, 128, 512
NCORES = 8
BLOCAL = BATCH // NCORES   # 2
L = 8                      # chunk length
NK = T // L                # 256 chunks per batch element
NKB = BLOCAL * NK          # 512 chunk-columns per core (b-major)
NT = N // 128              # 4 partition tiles over the state dim
COLS = BLOCAL * T          # 4096
W2 = 2 * NKB               # 1024: width of packed [r|i] phase-2 tiles

# blob (fp16) column layout:
#   u [0:4096] | Pt (4 x 2048) | K (1024) | tables (4 x 3072) | Qt (8192)
UW = L * NKB               # 4096
PW = 2 * L * 128           # 2048 per m  (ri-major, j-minor, 128 each)
KW = L * 128               # 1024
TRW = 3 * W2               # 3072 per m: ckck | skpm | rr
QW = L * 2 * NT * 128      # 8192
O_PT = UW
O_K = O_PT + NT * PW
O_TR = O_K + KW
O_QT = O_TR + NT * TRW
W16 = O_QT + QW            # 33792

LAST_RESULT = None
_NC_CACHE = None


def _build_nc():
    from concourse import bass, mybir
    from concourse import tile

    f32 = mybir.dt.float32
    f16 = mybir.dt.float16
    op = mybir.AluOpType

    nc = bass.Bass("TRN2", target_bir_lowering=False, debug=False)

    blob = nc.dram_tensor("blob", [128, W16], f16, kind="ExternalInput")
    yout = nc.dram_tensor("y", [OUT, COLS], f16, kind="ExternalOutput")

    with tile.TileContext(nc) as tc:
        with (
            tc.tile_pool(name="const", bufs=1) as cpool,
            tc.tile_pool(name="vsb", bufs=4) as vpool,
            tc.tile_pool(name="tmp", bufs=2) as tpool,
            tc.tile_pool(name="gz", bufs=2) as gpool,
            tc.tile_pool(name="S", bufs=1) as spool,
            tc.tile_pool(name="ysb", bufs=4) as ypool_sb,
            tc.tile_pool(name="ps", bufs=1, space="PSUM") as pspool,
        ):
            b16 = cpool.tile([128, W16], f16)

            u_j = [b16[:, j * NKB:(j + 1) * NKB] for j in range(L)]
            ptT = [[[None] * L for _ in range(2)] for _ in range(NT)]
            for m in range(NT):
                for ri in range(2):
                    for j in range(L):
                        o = O_PT + m * PW + ri * L * 128 + j * 128
                        ptT[m][ri][j] = b16[:, o:o + 128]
            ktT = [b16[:, O_K + d * 128:O_K + (d + 1) * 128] for d in range(L)]
            ckck = [None] * NT
            skpm = [None] * NT
            rr = [None] * NT
            for m in range(NT):
                o = O_TR + m * TRW
                ckck[m] = b16[:, o:o + W2]
                skpm[m] = b16[:, o + W2:o + 2 * W2]
                rr[m] = b16[:, o + 2 * W2:o + 3 * W2]
            qtT = [[[None] * NT for _ in range(2)] for _ in range(L)]
            for j in range(L):
                for ri in range(2):
                    for m in range(NT):
                        o = O_QT + (j * 2 + ri) * NT * 128 + m * 128
                        qtT[j][ri][m] = b16[:, o:o + 128]

            # ---- DMA: two HW rings only (the gpsimd ring is a slow SWDGE
            # path and 3-way queue splitting starves everyone). Triggers
            # cost ~0.7us each on the issuing engine, so few, large pieces:
            #   sync q:   u_j0 | Pt0 | u_rest | Pt1 | Pt2 | Pt3, y-out later
            #   scalar q: K, tr0 up front; tr1, Qt halves, tr2/tr3
            #             interleaved with the v-copies (just-in-time)
            def tr_dma(m, eng=None):
                lo, hi = O_TR + m * TRW, O_TR + (m + 1) * TRW
                (eng or nc.scalar).dma_start(b16[:, lo:hi], blob[:, lo:hi])

            # u split across BOTH rings to beat the DMA rate ramp; phase-1 m0
            # starts only when it can run gap-free (a stall-y early matmul
            # burst provokes a HAM half-speed window)
            nc.sync.dma_start(b16[:, 0:UW // 2], blob[:, 0:UW // 2])
            h0 = O_PT + PW // 2
            nc.sync.dma_start(b16[:, O_PT:h0], blob[:, O_PT:h0])
            nc.sync.dma_start(b16[:, h0:O_PT + PW], blob[:, h0:O_PT + PW])
            nc.scalar.dma_start(b16[:, UW // 2:UW], blob[:, UW // 2:UW])
            nc.scalar.dma_start(b16[:, O_K:O_K + KW], blob[:, O_K:O_K + KW])
            tr_dma(0)  # m0 tables: after K (needed ~17us, lands ~15)
            for m in range(1, NT):
                lo, hi = O_PT + m * PW, O_PT + (m + 1) * PW
                nc.sync.dma_start(b16[:, lo:hi], blob[:, lo:hi])

            # PE p-state warmup on a memset tile, entirely inside the DMA
            # dead zone (no data dependencies); output discarded.
            wsrc = vpool.tile([128, NKB], f16, tag="warm", name="wsrc", bufs=1)
            nc.vector.memset(wsrc[:], 0.0)
            wp = pspool.tile([128, NKB], f32, tag="vt", bufs=2, name="warm")
            for wi in range(14):
                nc.tensor.matmul(wp[:], wsrc[:, 0:128], wsrc[:],
                                 start=(wi == 0), stop=(wi == 13))

            v_t = [None] * NT
            S_t = [None] * NT
            yps = {}

            def phase1(m, mid=None):
                # ri-major with the v-copy issued right after each half, so
                # the real part of v reaches the DVE ~1.3us earlier; `mid`
                # emits PE filler (conv) between the halves to cover Pt DMA
                v = vpool.tile([128, W2], f16, tag="v", name=f"v{m}")
                for ri in range(2):
                    vt = pspool.tile([128, NKB], f32, tag="vt", bufs=2,
                                     name=f"vt{m}{ri}")
                    for j in range(L):
                        nc.tensor.matmul(
                            vt[:], ptT[m][ri][j], u_j[j],
                            start=(j == 0), stop=(j == L - 1),
                        )
                    nc.scalar.copy(v[:, ri * NKB:(ri + 1) * NKB], vt[:])
                    if ri == 0 and mid:
                        mid()
                v_t[m] = v

            def phase2(m, split_first=False):
                ta = tpool.tile([128, W2], f16, tag="ta", name=f"ta{m}")
                tb = tpool.tile([128, W2], f16, tag="tb", name=f"tb{m}")
                tc_ = tpool.tile([128, W2], f16, tag="tc", name=f"tc{m}")
                td = tpool.tile([128, W2], f16, tag="td", name=f"td{m}")
                g = gpool.tile([128, W2], f16, tag="g", name=f"g{m}")
                z = gpool.tile([128, W2], f16, tag="z", name=f"z{m}")
                S = spool.tile([128, W2], f16, tag=f"S{m}", name=f"S{m}")
                v = v_t[m]
                V = nc.vector
                if split_first:
                    # vr-dependent halves first: vr lands ~1.4us before vi
                    V.tensor_tensor(ta[:, 0:NKB], ckck[m][:, 0:NKB],
                                    v[:, 0:NKB], op=op.mult)
                    V.tensor_tensor(tb[:, NKB:W2], skpm[m][:, NKB:W2],
                                    v[:, 0:NKB], op=op.mult)
                    V.tensor_tensor(ta[:, NKB:W2], ckck[m][:, NKB:W2],
                                    v[:, NKB:W2], op=op.mult)
                    V.tensor_tensor(tb[:, 0:NKB], skpm[m][:, 0:NKB],
                                    v[:, NKB:W2], op=op.mult)
                else:
                    V.tensor_tensor(ta[:], ckck[m], v[:], op=op.mult)
                    # t_b = [sk*vi | -sk*vr] (crossed halves)
                    V.tensor_tensor(tb[:, 0:NKB], skpm[m][:, 0:NKB],
                                    v[:, NKB:W2], op=op.mult)
                    V.tensor_tensor(tb[:, NKB:W2], skpm[m][:, NKB:W2],
                                    v[:, 0:NKB], op=op.mult)
                V.tensor_tensor(g[:], ta[:], tb[:], op=op.add)
                V.tensor_tensor_scan(z[:], rr[m], g[:], 0.0,
                                     op0=op.mult, op1=op.add)
                if m == NT - 1:
                    # demod split r-half-first: Sr3 unblocks the final bnd
                    # block ~1us before Si3 is needed
                    V.tensor_tensor(tc_[:, 0:NKB], ckck[m][:, 0:NKB],
                                    z[:, 0:NKB], op=op.mult)
                    V.tensor_tensor(td[:, 0:NKB], skpm[m][:, NKB:W2],
                                    z[:, NKB:W2], op=op.mult)
                    V.tensor_tensor(S[:, 0:NKB], tc_[:, 0:NKB],
                                    td[:, 0:NKB], op=op.add)
                    V.tensor_tensor(tc_[:, NKB:W2], ckck[m][:, NKB:W2],
                                    z[:, NKB:W2], op=op.mult)
                    V.tensor_tensor(td[:, NKB:W2], skpm[m][:, 0:NKB],
                                    z[:, 0:NKB], op=op.mult)
                    V.tensor_tensor(S[:, NKB:W2], tc_[:, NKB:W2],
                                    td[:, NKB:W2], op=op.add)
                else:
                    V.tensor_tensor(tc_[:], ckck[m], z[:], op=op.mult)
                    # t_d = [-sk*zi | sk*zr] (crossed halves)
                    V.tensor_tensor(td[:, 0:NKB], skpm[m][:, NKB:W2],
                                    z[:, NKB:W2], op=op.mult)
                    V.tensor_tensor(td[:, NKB:W2], skpm[m][:, 0:NKB],
                                    z[:, 0:NKB], op=op.mult)
                    V.tensor_tensor(S[:], tc_[:], td[:], op=op.add)
                S_t[m] = S

            # bnd uses ONE 511-col matmul per (j,ri,m) spanning both batch
            # halves; the batch-boundary column (k=0 of b1) picks up a bogus
            # carry-over, so its conv-only value is captured here and patched
            # back after the eviction cast.
            ccol = vpool.tile([128, L], f32, tag="ccol", name="ccol", bufs=1)

            def capture(j):
                nc.scalar.copy(ccol[:, j:j + 1], yps[j][:, NK:NK + 1])

            def conv(j, tag="y", cap=True):
                yps[j] = pspool.tile([128, NKB], f32, tag=tag,
                                     bufs=(6 if tag == "y" else 2),
                                     name=f"y{j}")
                for d in range(j + 1):
                    nc.tensor.matmul(
                        yps[j][:], ktT[d], u_j[j - d],
                        start=(d == 0), stop=False, skip_group_check=True,
                    )
                if cap:
                    capture(j)

            # ---- emission: p1/conv interleaved so conv fills DMA bubbles;
            # conv4..7 deferred into the bnd section to cover S-latency ----
            qh = O_QT + QW // 2
            phase1(0, mid=lambda: (conv(0, cap=False), conv(1, cap=False)))
            phase2(0, split_first=True)
            tr_dma(1)
            nc.scalar.dma_start(b16[:, O_QT:qh], blob[:, O_QT:qh])  # Qt j0..3
            conv(2, cap=False)
            phase1(1, mid=lambda: conv(3, cap=False))
            phase2(1)
            nc.scalar.dma_start(b16[:, qh:W16], blob[:, qh:W16])    # Qt j4..7
            tr_dma(2)
            phase1(2, mid=lambda: conv(4, cap=False))
            phase2(2)
            tr_dma(3)
            phase1(3, mid=lambda: conv(5, cap=False))
            phase2(3)
            # captures for j0..5 AFTER the last v-copy: a capture queued
            # between v-copies on the scalar ring delays them (WAR on conv)
            # and fragments the whole DVE chain
            for j in range(6):
                capture(j)

            # ---- boundary matmuls, m-outer (S3 needed only at the end) ----
            def bnd(j, m, ri, stop=False):
                S = S_t[m]
                a0 = ri * NKB
                nc.tensor.matmul(
                    yps[j][:, 1:NKB], qtT[j][ri][m],
                    S[:, a0:a0 + NKB - 1], start=False, stop=stop,
                    skip_group_check=True,
                )

            conv(6, tag="vt")   # vt PSUM banks are free after p1(3)'s copies
            for m in range(NT - 1):
                for j in range(L - 1):
                    for ri in range(2):
                        bnd(j, m, ri)
            # j=7 column deferred: conv(7) + its m0..2 bnds sit right before
            # the m3 block, filling the wait for S3
            conv(7, tag="vt")
            for m in range(NT - 1):
                for ri in range(2):
                    bnd(L - 1, m, ri)
            # last m: real parts first (Si3 lands after Sr3), then imag
            # j-by-j with eviction casts streaming right behind into ONE
            # ysb tile; y-out DMA in 3 pieces so the transfers overlap the
            # remaining casts.
            ysb = ypool_sb.tile([128, COLS], f16, tag="ysb", bufs=1)
            # batch-boundary columns (conv-only) written into ysb EARLY,
            # off the tail; the eviction casts below skip that column
            for j in range(L):
                nc.scalar.copy(ysb[:, j * NKB + NK:j * NKB + NK + 1],
                               ccol[:, j:j + 1])
            for j in range(L):
                bnd(j, NT - 1, 0)
            for j in range(L):
                bnd(j, NT - 1, 1, stop=True)
                o = j * NKB
                # each eviction split across BOTH engines: halves the per-j
                # cast latency so the y-out pieces can launch earlier
                nc.scalar.copy(ysb[:, o:o + NK], yps[j][:, 0:NK])
                nc.vector.tensor_copy(ysb[:, o + NK + 1:o + NKB],
                                      yps[j][:, NK + 1:NKB])
                # y-out in 4 progressive pieces, all on the idle sync ring
                # (scalar-ring triggers would delay its cast halves): the
                # BW-bound 1MB write starts ~1.2us earlier
                if j == 1:
                    nc.sync.dma_start(yout[:, 0:2 * NKB], ysb[:, 0:2 * NKB])
                if j == 4:
                    nc.sync.dma_start(yout[:, 2 * NKB:5 * NKB],
                                      ysb[:, 2 * NKB:5 * NKB])
                if j == 6:
                    nc.sync.dma_start(yout[:, 5 * NKB:7 * NKB],
                                      ysb[:, 5 * NKB:7 * NKB])
            nc.sync.dma_start(yout[:, 7 * NKB:COLS], ysb[:, 7 * NKB:COLS])
    _legalize_multi_waits(nc)
    return nc


def _legalize_multi_waits(nc):
    """This walrus build accepts a single sync wait per instruction; split
    any multi-wait instruction into same-engine single-wait NoOps + the
    original carrying the last wait (program order chains them)."""
    import bass_rust
    from concourse import mybir

    uid = [0]
    for fn in nc.m.functions:
        for bb in fn.blocks:
            insts = bb.instructions
            new = []
            changed = False
            for inst in insts:
                si = inst.sync_info
                if si is not None and len(si.on_wait) > 1:
                    waits = list(si.on_wait)
                    for w in waits[:-1]:
                        uid[0] += 1
                        new.append(mybir.InstNoOp(
                            name=f"mwsplit-{uid[0]}",
                            engine=inst.engine,
                            ins=[], outs=[],
                            sync_info=bass_rust.SyncInfo(on_wait=[w], on_update=[]),
                        ))
                    inst.sync_info = bass_rust.SyncInfo(
                        on_wait=[waits[-1]], on_update=list(si.on_update)
                    )
                    changed = True
                new.append(inst)
            if changed:
                bb.instructions = new


def _host_prep(A_re, A_im, B_re, B_im, C_re, C_im, D_w):
    """fp64 eigendecomposition + chunked-formulation weight/table layouts.
    Returns shared fp16 tail of the blob: [128, W16 - UW]."""
    A = A_re.astype(np.float64) + 1j * A_im.astype(np.float64)
    w, V = np.linalg.eig(A)
    Vinv = np.linalg.inv(V)
    Bt = Vinv @ (B_re.astype(np.float64) + 1j * B_im.astype(np.float64))
    Ct = (C_re.astype(np.float64) + 1j * C_im.astype(np.float64)) @ V

    Pt = np.stack([(w ** (L - 1 - j))[:, None] * Bt for j in range(L)])  # [L,N,IN]
    Qt = np.stack([Ct * (w ** (j + 1))[None, :] for j in range(L)])      # [L,OUT,N]
    K = np.empty((L, OUT, IN))
    Ad = np.eye(N, dtype=complex)
    Bc = B_re.astype(np.float64) + 1j * B_im.astype(np.float64)
    Cc = C_re.astype(np.float64) + 1j * C_im.astype(np.float64)
    for d in range(L):
        K[d] = (Cc @ Ad @ Bc).real
        Ad = A @ Ad
    K[0] += D_w.astype(np.float64)

    wL = w ** L
    rhoL = np.abs(wL)
    phi = np.angle(wL)
    kk = np.arange(NK)
    cosk = np.cos(np.outer(phi, kk + 1))  # [N, NK]
    sink = np.sin(np.outer(phi, kk + 1))

    parts = []
    for m in range(NT):
        sl = slice(m * 128, (m + 1) * 128)
        for Pp in (Pt.real, Pt.imag):
            for j in range(L):
                parts.append(np.ascontiguousarray(Pp[j].T[:, sl]))  # [IN, 128]
    for d in range(L):
        parts.append(np.ascontiguousarray(K[d].T))  # [IN, OUT]
    for m in range(NT):
        sl = slice(m * 128, (m + 1) * 128)
        ck = np.tile(cosk[sl], (1, BLOCAL))          # [128, NKB]
        sk = np.tile(sink[sl], (1, BLOCAL))
        rb = np.broadcast_to(rhoL[sl][:, None], (128, NKB)).copy()
        rb[:, NK] = 0.0  # reset scan state at second batch element
        rr2 = np.concatenate([rb, rb], axis=1)       # [128, W2]
        rr2[:, NKB] = 0.0  # reset crossing into the imag half
        parts.append(np.concatenate([ck, ck], axis=1))   # ckck
        parts.append(np.concatenate([sk, -sk], axis=1))  # skpm
        parts.append(rr2)                                # rr
    for j in range(L):
        for Qp in (Qt[j].real, -Qt[j].imag):
            QT = np.ascontiguousarray(Qp.T)  # [N, OUT]
            for m in range(NT):
                parts.append(QT[m * 128:(m + 1) * 128])
    shared = np.concatenate(parts, axis=1).astype(np.float16)
    assert shared.shape == (128, W16 - UW)
    return shared


def _ensure_axon_hooks():
    """Provide antenv.axon_hooks if the image lacks it (needed only for
    trace=True NTFF profiling; run path works without)."""
    import types
    try:
        from antenv import axon_hooks  # noqa: F401
        return
    except ImportError:
        pass
    try:
        import antenv
        mod = types.ModuleType("antenv.axon_hooks")
        _hook = [None]
        mod.set_axon_ntff_profile_hook = lambda h: _hook.__setitem__(0, h)
        mod.get_axon_ntff_profile_hook = lambda: _hook[0]
        sys.modules["antenv.axon_hooks"] = mod
        antenv.axon_hooks = mod
        if "/root/.axon_site" not in sys.path:
            sys.path.insert(0, "/root/.axon_site")
        from trn_agent_boot.trn_boot import _ntff_profile_via_ctypes
        h = _ntff_profile_via_ctypes("/opt/axon/libaxon_pjrt.so")
        if h is not None:
            mod.set_axon_ntff_profile_hook(h)
    except Exception:
        pass


def kernel(u, A_re, A_im, B_re, B_im, C_re, C_im, D_w, output_bias):
    global LAST_RESULT, _NC_CACHE
    from concourse import bass_utils

    _ensure_axon_hooks()

    u = np.asarray(u, dtype=np.float32)
    shared = _host_prep(
        np.asarray(A_re), np.asarray(A_im), np.asarray(B_re), np.asarray(B_im),
        np.asarray(C_re), np.asarray(C_im), np.asarray(D_w)
    )

    if _NC_CACHE is None:
        _NC_CACHE = _build_nc()
    nc = _NC_CACHE

    in_maps = []
    for c in range(NCORES):
        up = u[BLOCAL * c:BLOCAL * (c + 1)]           # [2, T, IN]
        uc = up.reshape(BLOCAL, NK, L, IN)            # t = k*L + j
        u_jk = np.ascontiguousarray(
            uc.transpose(3, 2, 0, 1).reshape(IN, L * NKB)
        ).astype(np.float16)                          # col = j*NKB + b*NK + k
        in_maps.append({"blob": np.concatenate([u_jk, shared], axis=1)})

    res = bass_utils.run_bass_kernel_spmd(nc, in_maps, core_ids=list(range(NCORES)))
    LAST_RESULT = res

    y = np.empty((BATCH, T, OUT), dtype=np.float32)
    for c in range(NCORES):
        yd = np.asarray(res.results[c]["y"], dtype=np.float32)  # [OUT, L*NKB]
        y[BLOCAL * c:BLOCAL * (c + 1)] = (
            yd.reshape(OUT, L, BLOCAL, NK).transpose(2, 3, 1, 0)
            .reshape(BLOCAL, T, OUT)
        )
    y += np.asarray(output_bias, dtype=np.float32)
    return y
